# revision 15
# baseline (speedup 1.0000x reference)
"""Trainium2 Bass kernel for a 2-layer GCN (nn_EvenLamerGCN) — v3.

reference semantics (PyG GCNConv x2, eval mode):
    deg[i]  = 1 + indeg(i)                (self-loops added)
    dinv    = deg ** -0.5
    h  = relu(A_hat @ (x @ W1) + b1),  A_hat = D^-1/2 (A + I) D^-1/2
    o  = A_hat @ (h @ W2) + b2
    return o, log_softmax(o, axis=1)

v3 = v1 (SWDGE dma_gather rows + one-hot scatter matmuls) with:
  * 64-granular cell quotas (was 128-chunk granularity): per-(window,
    block) slot quotas are mult-of-64 maxima over cores; chunks of 128
    slots may straddle two cells, handled by partial-row matmuls
    (base partition 0/64) and parity-offset slot ids (S is built
    256 wide; even blocks match columns 0-127, odd blocks 128-255).
  * 4 equal-ish windows defined by LOCAL row quarters, each with its
    own t_loc/t_full tensors and its own AllGather, so collectives
    pipeline against phase-0 tails and gather processing.
  * self-loops seeded into acc from the local table shard (no
    descriptors).
"""

import sys

for _p in ("/opt/trn_rl_repo", "/root/.axon_site/_ro/trn_rl_repo"):
    if _p not in sys.path:
        sys.path.insert(0, _p)

from contextlib import ExitStack
from dataclasses import dataclass

import numpy as np

import concourse.bass as bass
import concourse.mybir as mybir
import concourse.tile as tile
from concourse import bacc
from concourse.bass import ds, ts
from concourse.bass_utils import run_bass_kernel_spmd
from concourse.masks import make_identity

F32 = mybir.dt.float32
BF16 = mybir.dt.bfloat16
I16 = mybir.dt.int16
AF = mybir.ActivationFunctionType
ALU = mybir.AluOpType

MAXP = 32            # chunks per gather instruction


@dataclass(frozen=True)
class Cfg:
    n: int = 100000
    din: int = 512
    dh: int = 128
    dout: int = 40
    cores: int = 8

    @property
    def nsh(self):
        return self.n // self.cores

    @property
    def nloc(self):
        return ((self.nsh + 127) // 128) * 128

    @property
    def nt(self):
        return self.nloc // 128

    @property
    def dh2(self):
        return 64

    @property
    def dt2(self):
        return 128

    @property
    def kt(self):
        return self.din // 128

    @property
    def qtiles(self):
        # phase-0 tile counts per window-quarter (sums to nt)
        q, r = divmod(self.nt, 4)
        return tuple(q + (1 if i < r else 0) for i in range(4))

    @property
    def qstart(self):
        s, out = 0, []
        for q in self.qtiles:
            out.append(s)
            s += q
        return tuple(out)

    @property
    def qrows(self):
        return tuple(q * 128 for q in self.qtiles)


@dataclass(frozen=True)
class Plan:
    q: tuple          # [w][b] slot quota, mult of 64
    nchunks: tuple    # chunks per window
    chunk_ops: tuple  # [w][chunk] -> ((b, k0, k1, start, stop), ...)

    @property
    def total_chunks(self):
        return sum(self.nchunks)


def make_plan(cfg: Cfg, counts: np.ndarray) -> Plan:
    c = cfg
    W = len(c.qtiles)
    qmax = counts.max(axis=0)                # [W, nt]
    q = ((qmax + 63) // 64) * 64
    nchunks = []
    chunk_ops = []
    for w in range(W):
        sw = int(q[w].sum())
        nch = (sw + 127) // 128
        nchunks.append(nch)
        ops = [[] for _ in range(nch)]
        cur = 0
        ordinal = 0
        for b in range(c.nt):
            qb = int(q[w][b])
            if qb == 0:
                continue
            par = ordinal & 1
            ordinal += 1
            lo, hi = cur, cur + qb
            cur = hi
            c0, c1 = lo // 128, (hi - 1) // 128
            for ci in range(c0, c1 + 1):
                k0 = max(lo, ci * 128) - ci * 128
                k1 = min(hi, (ci + 1) * 128) - ci * 128
                assert k0 in (0, 64), (w, b, lo, hi, ci)
                ops[ci].append((b, par, k0, k1, ci == c0, ci == c1))
        chunk_ops.append(tuple(tuple(x) for x in ops))
    return Plan(
        q=tuple(tuple(int(v) for v in row) for row in q),
        nchunks=tuple(nchunks),
        chunk_ops=tuple(chunk_ops),
    )


def preprocess(cfg: Cfg, edge_index: np.ndarray):
    c = cfg
    W = len(c.qtiles)
    src = np.asarray(edge_index[0], dtype=np.int64)
    dst = np.asarray(edge_index[1], dtype=np.int64)

    deg = np.bincount(dst, minlength=c.n).astype(np.float32) + 1.0
    deg_pt = np.ones((c.cores, 128, c.nt), np.float32)
    for ci in range(c.cores):
        dl = np.ones(c.nloc, np.float32)
        dl[: c.nsh] = deg[ci * c.nsh : (ci + 1) * c.nsh]
        deg_pt[ci] = dl.reshape(c.nt, 128).T

    core = dst // c.nsh
    dloc = dst - core * c.nsh
    b_all = dloc // 128
    id_all = dloc % 128
    csrc = src // c.nsh
    sloc = src - csrc * c.nsh
    qstart_rows = np.array([t * 128 for t in c.qstart] + [c.nloc], np.int64)
    qrows = np.array(c.qrows, np.int64)
    w_all = np.searchsorted(qstart_rows, sloc, side="right") - 1
    r_all = csrc * qrows[w_all] + (sloc - qstart_rows[w_all])

    cell_key = (core * W + w_all) * c.nt + b_all
    counts = np.bincount(cell_key, minlength=c.cores * W * c.nt)
    counts = counts.reshape(c.cores, W, c.nt)
    plan = make_plan(c, counts)

    tot_slots = plan.total_chunks * 128
    TC = plan.total_chunks

    idx16 = np.zeros((c.cores, 128, tot_slots // 16), np.int16)
    sidf = np.empty((c.cores, 128, TC), np.float32)

    order = np.lexsort((r_all, b_all, w_all, core))
    so_r, so_b, so_id = r_all[order], b_all[order], id_all[order]
    so_core = core[order]
    core_starts = np.searchsorted(so_core, np.arange(c.cores + 1))

    wbase = np.concatenate([[0], np.cumsum([n * 128 for n in plan.nchunks])])
    cell_off = np.zeros((W, c.nt), np.int64)
    cell_par = np.zeros((W, c.nt), np.int64)
    for w in range(W):
        cur = 0
        ordinal = 0
        for b in range(c.nt):
            cell_off[w][b] = cur
            if plan.q[w][b]:
                cell_par[w][b] = ordinal & 1
                ordinal += 1
            cur += plan.q[w][b]

    for ci in range(c.cores):
        lo, hi = core_starts[ci], core_starts[ci + 1]
        rr, ii = so_r[lo:hi], so_id[lo:hi]
        vals = np.zeros(tot_slots, np.int64)
        sids = np.full(tot_slots, -1.0, np.float32)
        cnt = counts[ci]
        pos = 0
        for w in range(W):
            for b in range(c.nt):
                n = cnt[w][b]
                if n:
                    off = wbase[w] + cell_off[w][b]
                    vals[off : off + n] = rr[pos : pos + n]
                    sids[off : off + n] = ii[pos : pos + n] + 128 * cell_par[w][b]
                    pos += n
        assert pos == hi - lo
        v = vals.reshape(-1, 16)
        idx16[ci] = np.tile(np.ascontiguousarray(v.T), (8, 1)).astype(np.int16)
        sidf[ci] = sids.reshape(TC, 128).T

    return deg_pt, idx16, sidf, plan


# ----------------------------------------------------------------------------
# Device kernel
# ----------------------------------------------------------------------------

def build(nc, tc, cfg: Cfg, plan: Plan):
    c = cfg
    RG = [list(range(c.cores))]
    W = len(c.qtiles)
    TC = plan.total_chunks
    tot_slots = TC * 128
    maxch = max(plan.nchunks)

    x_sh = nc.dram_tensor("x_sh", [c.nloc, c.din], BF16, kind="ExternalInput").ap()
    w1 = nc.dram_tensor("w1", [c.din, c.dh], BF16, kind="ExternalInput").ap()
    w2 = nc.dram_tensor("w2", [c.dh, c.dh2], F32, kind="ExternalInput").ap()
    b1r = nc.dram_tensor("b1r", [128, c.dh], F32, kind="ExternalInput").ap()
    b2r = nc.dram_tensor("b2r", [128, c.dh2], F32, kind="ExternalInput").ap()
    degp = nc.dram_tensor("degp", [128, c.nt], F32, kind="ExternalInput").ap()
    idx16 = nc.dram_tensor("idx16", [128, tot_slots // 16], I16,
                           kind="ExternalInput").ap()
    idsf = nc.dram_tensor("idsf", [128, TC], BF16, kind="ExternalInput").ap()
    out_h = nc.dram_tensor("out_h", [c.nloc, c.dh2], F32, kind="ExternalOutput").ap()
    out_ls = nc.dram_tensor("out_ls", [c.nloc, c.dh2], F32, kind="ExternalOutput").ap()

    t1_loc = [
        nc.dram_tensor(f"t1_loc{j}", [c.qrows[j], c.dh], BF16, kind="Internal").ap()
        for j in range(W)
    ]
    t1_full = [
        nc.dram_tensor(f"t1_full{j}", [c.cores * c.qrows[j], c.dh], BF16,
                       kind="Internal", addr_space="Shared").ap()
        for j in range(W)
    ]
    t2_loc = [
        nc.dram_tensor(f"t2_loc{j}", [c.qrows[j], c.dt2], BF16, kind="Internal").ap()
        for j in range(W)
    ]
    t2_full = [
        nc.dram_tensor(f"t2_full{j}", [c.cores * c.qrows[j], c.dt2], BF16,
                       kind="Internal", addr_space="Shared").ap()
        for j in range(W)
    ]

    with ExitStack() as st:
        cpool = st.enter_context(tc.tile_pool(name="consts", bufs=1))
        accp = st.enter_context(tc.tile_pool(name="acc", bufs=1))
        gp = st.enter_context(tc.tile_pool(name="gp", bufs=2))
        sp = st.enter_context(tc.tile_pool(name="sp", bufs=2))
        pp = st.enter_context(tc.tile_pool(name="pp", bufs=2))
        ppsum = st.enter_context(tc.tile_pool(name="ppsum", bufs=4, space="PSUM"))
        p0 = st.enter_context(tc.tile_pool(name="p0", bufs=3))
        p0ps = st.enter_context(tc.tile_pool(name="p0ps", bufs=2, space="PSUM"))
        p0psT = st.enter_context(tc.tile_pool(name="p0psT", bufs=2, space="PSUM"))

        # ---- constants ----
        ident = cpool.tile([128, 128], F32)
        make_identity(nc, ident)
        identb = cpool.tile([128, 128], BF16)
        make_identity(nc, identb)
        w1sb = cpool.tile([128, c.kt, c.dh], BF16)
        nc.sync.dma_start(w1sb, w1.rearrange("(o p) f -> p o f", p=128))
        w2sb = cpool.tile([128, c.dh2], F32)
        nc.sync.dma_start(w2sb, w2)
        b1sb = cpool.tile([128, c.dh], F32)
        nc.sync.dma_start(b1sb, b1r)
        b2sb = cpool.tile([128, c.dh2], F32)
        nc.sync.dma_start(b2sb, b2r)
        dinv = cpool.tile([128, c.nt], F32)
        nc.sync.dma_start(dinv, degp)
        nc.scalar.activation(dinv, dinv, AF.Sqrt)
        nc.vector.reciprocal(dinv, dinv)
        iota = cpool.tile([128, MAXP, 256], BF16)
        nc.gpsimd.iota(iota, pattern=[[0, MAXP], [1, 256]], base=0,
                       channel_multiplier=0,
                       allow_small_or_imprecise_dtypes=True)

        # ---- phase 0: T1 = dinv * (x @ W1), quarter AllGathers pipelined ----
        for w in range(W):
            for tt in range(c.qtiles[w]):
                t = c.qstart[w] + tt
                xt = p0.tile([128, c.din], BF16, tag="xt")
                nc.sync.dma_start(xt, x_sh[ts(t, 128), :])
                hps = p0ps.tile([128, c.dh], F32, tag="hps")
                for j in range(c.kt):
                    tps = p0psT.tile([128, 128], BF16, tag="tps")
                    nc.tensor.transpose(tps, xt[:, ts(j, 128)], identb)
                    xT = p0.tile([128, 128], BF16, tag="xT")
                    nc.vector.tensor_copy(xT, tps)
                    nc.tensor.matmul(
                        hps, lhsT=xT, rhs=w1sb[:, j, :],
                        start=(j == 0), stop=(j == c.kt - 1),
                    )
                hsb = p0.tile([128, c.dh], BF16, tag="hsb")
                nc.vector.tensor_scalar_mul(hsb, hps, dinv[:, t : t + 1])
                nc.sync.dma_start(t1_loc[w][ts(tt, 128), :], hsb)
            nc.gpsimd.collective_compute(
                "AllGather", ALU.bypass, replica_groups=RG,
                ins=[t1_loc[w].opt()], outs=[t1_full[w].opt()],
            )

        # ---- edge aggregation ----
        def edge_phase(tfull_list, tloc_list, acc, d, dt):
            # self-loop seed: acc[:, b, :] = T[b-th block rows]
            for w in range(W):
                tv = tloc_list[w].rearrange("(b p) f -> p b f", p=128)
                for tt in range(c.qtiles[w]):
                    sd = p0.tile([128, dt], BF16, tag="sd")
                    nc.sync.dma_start(sd[:, :d], tv[:, tt, :d])
                    nc.vector.tensor_copy(acc[:, c.qstart[w] + tt, :], sd[:, :d])
            chunk0 = 0
            psum_open = {}
            for w in range(W):
                nch_w = plan.nchunks[w]
                sit = sp.tile([128, maxch * 8], I16, tag="sit")
                nc.sync.dma_start(
                    sit[:, : nch_w * 8],
                    idx16[:, chunk0 * 8 : (chunk0 + nch_w) * 8],
                )
                sid = sp.tile([128, maxch], BF16, tag="sid")
                nc.sync.dma_start(sid[:, :nch_w], idsf[:, chunk0 : chunk0 + nch_w])
                loc = 0
                while loc < nch_w:
                    nch = min(MAXP, nch_w - loc)
                    g = gp.tile([128, MAXP, dt], BF16, tag="gt")
                    nc.gpsimd.dma_gather(
                        g[:, :nch, :], tfull_list[w],
                        sit[:, loc * 8 : (loc + nch) * 8],
                        num_idxs=nch * 128, num_idxs_reg=nch * 128, elem_size=dt,
                        single_packet=False, queue_num=3,
                    )
                    stt = pp.tile([128, MAXP, 256], BF16, tag="stt")
                    nc.vector.tensor_tensor(
                        stt[:, :nch, :], iota[:, :nch, :],
                        sid[:, loc : loc + nch, None].to_broadcast((128, nch, 256)),
                        ALU.is_equal,
                    )
                    for j in range(nch):
                        for (b, par, k0, k1, start, stop) in plan.chunk_ops[w][loc + j]:
                            if start:
                                pbt = ppsum.tile([128, d], F32, tag="ps")
                                psum_open[b] = pbt
                            nc.tensor.matmul(
                                psum_open[b],
                                lhsT=stt[k0:k1, j, ds(par * 128, 128)],
                                rhs=g[k0:k1, j, :d],
                                start=start, stop=stop,
                            )
                            if stop:
                                nc.vector.tensor_tensor(
                                    acc[:, b, :], acc[:, b, :], psum_open[b],
                                    ALU.add,
                                )
                                del psum_open[b]
                    loc += nch
                chunk0 += nch_w
            assert not psum_open

        acc1 = accp.tile([128, c.nt, c.dh], F32)
        edge_phase(t1_full, t1_loc, acc1, c.dh, c.dh)

        # ---- g1 = dinv * relu(dinv * agg + b1), in place, batched ----
        dinv_bc1 = dinv[:, :, None].to_broadcast((128, c.nt, c.dh))
        nc.vector.tensor_tensor(acc1, acc1, dinv_bc1, ALU.mult)
        nc.vector.tensor_tensor(
            acc1, acc1, b1sb[:, None, :].to_broadcast((128, c.nt, c.dh)), ALU.add
        )
        nc.scalar.activation(acc1, acc1, AF.Relu)
        nc.vector.tensor_tensor(acc1, acc1, dinv_bc1, ALU.mult)

        # ---- phase 2: T2 = g1 @ W2, quarter AllGathers pipelined ----
        for w in range(W):
            for tt in range(c.qtiles[w]):
                t = c.qstart[w] + tt
                tps = p0psT.tile([128, 128], F32, tag="tps")
                nc.tensor.transpose(tps, acc1[:, t, :], ident)
                gT = p0.tile([128, 128], F32, tag="gT")
                nc.vector.tensor_copy(gT, tps)
                h2ps = p0ps.tile([128, c.dh2], F32, tag="hps")
                nc.tensor.matmul(h2ps, lhsT=gT, rhs=w2sb, start=True, stop=True)
                h2sb = p0.tile([128, c.dh2], BF16, tag="h2sb")
                nc.vector.tensor_copy(h2sb, h2ps)
                nc.sync.dma_start(t2_loc[w][ts(tt, 128), : c.dh2], h2sb)
                nc.sync.dma_start(t2_loc[w][ts(tt, 128), c.dh2 :], h2sb)
            nc.gpsimd.collective_compute(
                "AllGather", ALU.bypass, replica_groups=RG,
                ins=[t2_loc[w].opt()], outs=[t2_full[w].opt()],
            )

        # ---- layer-2 edge aggregation ----
        acc2 = accp.tile([128, c.nt, c.dh2], F32)
        edge_phase(t2_full, t2_loc, acc2, c.dh2, c.dt2)

        # ---- h = dinv * agg2 + b2 ; log_softmax (batched) ----
        ohv = out_h.rearrange("(t p) f -> p t f", p=128)
        olv = out_ls.rearrange("(t p) f -> p t f", p=128)
        nc.vector.tensor_tensor(
            acc2, acc2, dinv[:, :, None].to_broadcast((128, c.nt, c.dh2)), ALU.mult
        )
        nc.vector.tensor_tensor(
            acc2, acc2, b2sb[:, None, :].to_broadcast((128, c.nt, c.dh2)), ALU.add
        )
        nc.sync.dma_start(ohv, acc2)
        accN = acc2[:, :, : c.dout]
        mx = accp.tile([128, c.nt], F32, tag="mx")
        nc.vector.tensor_reduce(mx, accN, mybir.AxisListType.X, ALU.max)
        nc.vector.tensor_tensor(
            accN, accN, mx[:, :, None].to_broadcast((128, c.nt, c.dout)),
            ALU.subtract,
        )
        e1 = accp.tile([128, c.nt, c.dout], F32, tag="e1")
        nc.scalar.activation(e1, accN, AF.Exp)
        se = accp.tile([128, c.nt], F32, tag="se")
        nc.vector.tensor_reduce(se, e1, mybir.AxisListType.X, ALU.add)
        ln = accp.tile([128, c.nt], F32, tag="ln")
        nc.scalar.activation(ln, se, AF.Ln)
        nc.vector.tensor_tensor(
            accN, accN, ln[:, :, None].to_broadcast((128, c.nt, c.dout)),
            ALU.subtract,
        )
        nc.sync.dma_start(olv[:, :, : c.dout], accN)


# ----------------------------------------------------------------------------
# Host entry point
# ----------------------------------------------------------------------------

_CACHE = {}


def _get_compiled(cfg: Cfg, plan: Plan):
    key = (cfg, plan)
    if key not in _CACHE:
        nc = bacc.Bacc(
            "TRN2", target_bir_lowering=False, debug=False,
            num_devices=cfg.cores, num_swdge_queues=4,
        )
        with tile.TileContext(nc) as tc:
            build(nc, tc, cfg, plan)
        nc.compile()
        _CACHE[key] = nc
    return _CACHE[key]


def make_in_maps(cfg: Cfg, x, W1, b1, W2, b2, deg_pt, idx16, sidf):
    import ml_dtypes

    c = cfg
    x = np.asarray(x, np.float32)
    w2p = np.zeros((c.dh, c.dh2), np.float32)
    w2p[:, : c.dout] = np.asarray(W2, np.float32)
    b1rep = np.tile(np.asarray(b1, np.float32)[None, :], (128, 1))
    b2p = np.zeros(c.dh2, np.float32)
    b2p[: c.dout] = np.asarray(b2, np.float32)
    b2rep = np.tile(b2p[None, :], (128, 1))
    w1c = np.ascontiguousarray(
        np.asarray(W1, np.float32).astype(ml_dtypes.bfloat16)
    )

    in_maps = []
    for ci in range(c.cores):
        xs = np.zeros((c.nloc, c.din), ml_dtypes.bfloat16)
        xs[: c.nsh] = x[ci * c.nsh : (ci + 1) * c.nsh].astype(ml_dtypes.bfloat16)
        in_maps.append({
            "x_sh": xs,
            "w1": w1c,
            "w2": w2p,
            "b1r": b1rep,
            "b2r": b2rep,
            "degp": np.ascontiguousarray(deg_pt[ci]),
            "idx16": np.ascontiguousarray(idx16[ci]),
            "idsf": np.ascontiguousarray(sidf[ci].astype(ml_dtypes.bfloat16)),
        })
    return in_maps


def _ensure_ntff_hook():
    """Install the axon NTFF profile hook if the image's antenv lacks it."""
    import types

    try:
        from antenv.axon_hooks import get_axon_ntff_profile_hook  # noqa: F401
        return
    except ImportError:
        pass
    import antenv

    m = types.ModuleType("antenv.axon_hooks")
    m._hook = None
    m.set_axon_ntff_profile_hook = lambda h: setattr(m, "_hook", h)
    m.get_axon_ntff_profile_hook = lambda: m._hook
    sys.modules["antenv.axon_hooks"] = m
    antenv.axon_hooks = m
    try:
        from trn_agent_boot.trn_boot import _ntff_profile_via_ctypes

        h = _ntff_profile_via_ctypes("/opt/axon/libaxon_pjrt.so")
        if h is not None:
            m._hook = h
    except Exception as e:
        print(f"ntff hook install failed: {e}")

    from concourse import bass_utils as bu

    bu.upload_artifacts = lambda tmpdir: tmpdir


def run(cfg: Cfg, inputs: dict, trace: bool = False):
    if trace:
        _ensure_ntff_hook()
    deg_pt, idx16, sidf, plan = preprocess(cfg, inputs["edge_index"])
    nc = _get_compiled(cfg, plan)
    in_maps = make_in_maps(
        cfg, inputs["x"], inputs["W1"], inputs["b1"], inputs["W2"], inputs["b2"],
        deg_pt, idx16, sidf,
    )
    res = run_bass_kernel_spmd(
        nc, in_maps, core_ids=list(range(cfg.cores)), trace=trace
    )
    c = cfg
    h = np.concatenate(
        [res.results[ci]["out_h"][: c.nsh, : c.dout] for ci in range(c.cores)],
        axis=0,
    )
    ls = np.concatenate(
        [res.results[ci]["out_ls"][: c.nsh, : c.dout] for ci in range(c.cores)],
        axis=0,
    )
    return (h, ls), res


def kernel(**inputs):
    (h, ls), _ = run(Cfg(), inputs)
    return h, ls


# revision 16
# speedup vs baseline: 1.1821x; 1.1821x over previous
"""Trainium2 Bass kernel for a 2-layer GCN (nn_EvenLamerGCN) — v3.

reference semantics (PyG GCNConv x2, eval mode):
    deg[i]  = 1 + indeg(i)                (self-loops added)
    dinv    = deg ** -0.5
    h  = relu(A_hat @ (x @ W1) + b1),  A_hat = D^-1/2 (A + I) D^-1/2
    o  = A_hat @ (h @ W2) + b2
    return o, log_softmax(o, axis=1)

v3 = v1 (SWDGE dma_gather rows + one-hot scatter matmuls) with:
  * 64-granular cell quotas (was 128-chunk granularity): per-(window,
    block) slot quotas are mult-of-64 maxima over cores; chunks of 128
    slots may straddle two cells, handled by partial-row matmuls
    (base partition 0/64) and parity-offset slot ids (S is built
    256 wide; even blocks match columns 0-127, odd blocks 128-255).
  * 4 equal-ish windows defined by LOCAL row quarters, each with its
    own t_loc/t_full tensors and its own AllGather, so collectives
    pipeline against phase-0 tails and gather processing.
  * self-loops seeded into acc from the local table shard (no
    descriptors).
"""

import sys

for _p in ("/opt/trn_rl_repo", "/root/.axon_site/_ro/trn_rl_repo"):
    if _p not in sys.path:
        sys.path.insert(0, _p)

from contextlib import ExitStack
from dataclasses import dataclass

import numpy as np

import concourse.bass as bass
import concourse.mybir as mybir
import concourse.tile as tile
from concourse import bacc
from concourse.bass import ds, ts
from concourse.bass_utils import run_bass_kernel_spmd
from concourse.masks import make_identity

F32 = mybir.dt.float32
BF16 = mybir.dt.bfloat16
I16 = mybir.dt.int16
AF = mybir.ActivationFunctionType
ALU = mybir.AluOpType

MAXP = 32            # chunks per gather instruction


@dataclass(frozen=True)
class Cfg:
    n: int = 100000
    din: int = 512
    dh: int = 128
    dout: int = 40
    cores: int = 8

    @property
    def nsh(self):
        return self.n // self.cores

    @property
    def nloc(self):
        return ((self.nsh + 127) // 128) * 128

    @property
    def nt(self):
        return self.nloc // 128

    @property
    def dh2(self):
        return 64

    @property
    def dt2(self):
        return 128

    @property
    def kt(self):
        return self.din // 128

    @property
    def qtiles(self):
        # phase-0 tile counts per window-quarter (sums to nt)
        q, r = divmod(self.nt, 4)
        return tuple(q + (1 if i < r else 0) for i in range(4))

    @property
    def qstart(self):
        s, out = 0, []
        for q in self.qtiles:
            out.append(s)
            s += q
        return tuple(out)

    @property
    def qrows(self):
        return tuple(q * 128 for q in self.qtiles)


@dataclass(frozen=True)
class Plan:
    q: tuple          # [w][b] slot quota, mult of 64
    nchunks: tuple    # chunks per window
    chunk_ops: tuple  # [w][chunk] -> ((b, k0, k1, start, stop), ...)

    @property
    def total_chunks(self):
        return sum(self.nchunks)


def make_plan(cfg: Cfg, counts: np.ndarray) -> Plan:
    c = cfg
    W = len(c.qtiles)
    qmax = counts.max(axis=0)                # [W, nt]
    q = ((qmax + 63) // 64) * 64
    nchunks = []
    chunk_ops = []
    for w in range(W):
        sw = int(q[w].sum())
        nch = (sw + 127) // 128
        nchunks.append(nch)
        ops = [[] for _ in range(nch)]
        cur = 0
        ordinal = 0
        for b in range(c.nt):
            qb = int(q[w][b])
            if qb == 0:
                continue
            par = ordinal & 1
            ordinal += 1
            lo, hi = cur, cur + qb
            cur = hi
            c0, c1 = lo // 128, (hi - 1) // 128
            for ci in range(c0, c1 + 1):
                k0 = max(lo, ci * 128) - ci * 128
                k1 = min(hi, (ci + 1) * 128) - ci * 128
                assert k0 in (0, 64), (w, b, lo, hi, ci)
                ops[ci].append((b, par, k0, k1, ci == c0, ci == c1))
        chunk_ops.append(tuple(tuple(x) for x in ops))
    return Plan(
        q=tuple(tuple(int(v) for v in row) for row in q),
        nchunks=tuple(nchunks),
        chunk_ops=tuple(chunk_ops),
    )


def preprocess(cfg: Cfg, edge_index: np.ndarray):
    c = cfg
    W = len(c.qtiles)
    src = np.asarray(edge_index[0], dtype=np.int64)
    dst = np.asarray(edge_index[1], dtype=np.int64)

    deg = np.bincount(dst, minlength=c.n).astype(np.float32) + 1.0
    deg_pt = np.ones((c.cores, 128, c.nt), np.float32)
    for ci in range(c.cores):
        dl = np.ones(c.nloc, np.float32)
        dl[: c.nsh] = deg[ci * c.nsh : (ci + 1) * c.nsh]
        deg_pt[ci] = dl.reshape(c.nt, 128).T

    core = dst // c.nsh
    dloc = dst - core * c.nsh
    b_all = dloc // 128
    id_all = dloc % 128
    csrc = src // c.nsh
    sloc = src - csrc * c.nsh
    w_all = csrc // 2
    r_all = (csrc % 2) * c.nloc + sloc

    cell_key = (core * W + w_all) * c.nt + b_all
    counts = np.bincount(cell_key, minlength=c.cores * W * c.nt)
    counts = counts.reshape(c.cores, W, c.nt)
    plan = make_plan(c, counts)

    tot_slots = plan.total_chunks * 128
    TC = plan.total_chunks

    idx16 = np.zeros((c.cores, 128, tot_slots // 16), np.int16)
    sidf = np.empty((c.cores, 128, TC), np.float32)

    order = np.lexsort((r_all, b_all, w_all, core))
    so_r, so_b, so_id = r_all[order], b_all[order], id_all[order]
    so_core = core[order]
    core_starts = np.searchsorted(so_core, np.arange(c.cores + 1))

    wbase = np.concatenate([[0], np.cumsum([n * 128 for n in plan.nchunks])])
    cell_off = np.zeros((W, c.nt), np.int64)
    cell_par = np.zeros((W, c.nt), np.int64)
    for w in range(W):
        cur = 0
        ordinal = 0
        for b in range(c.nt):
            cell_off[w][b] = cur
            if plan.q[w][b]:
                cell_par[w][b] = ordinal & 1
                ordinal += 1
            cur += plan.q[w][b]

    for ci in range(c.cores):
        lo, hi = core_starts[ci], core_starts[ci + 1]
        rr, ii = so_r[lo:hi], so_id[lo:hi]
        vals = np.zeros(tot_slots, np.int64)
        sids = np.full(tot_slots, -1.0, np.float32)
        cnt = counts[ci]
        pos = 0
        for w in range(W):
            for b in range(c.nt):
                n = cnt[w][b]
                if n:
                    off = wbase[w] + cell_off[w][b]
                    vals[off : off + n] = rr[pos : pos + n]
                    sids[off : off + n] = ii[pos : pos + n] + 128 * cell_par[w][b]
                    pos += n
        assert pos == hi - lo
        v = vals.reshape(-1, 16)
        idx16[ci] = np.tile(np.ascontiguousarray(v.T), (8, 1)).astype(np.int16)
        sidf[ci] = sids.reshape(TC, 128).T

    return deg_pt, idx16, sidf, plan


# ----------------------------------------------------------------------------
# Device kernel
# ----------------------------------------------------------------------------

def build(nc, tc, cfg: Cfg, plan: Plan):
    c = cfg
    RG = [list(range(c.cores))]
    W = len(c.qtiles)
    TC = plan.total_chunks
    tot_slots = TC * 128
    maxch = max(plan.nchunks)

    x_sh = nc.dram_tensor("x_sh", [c.nloc, c.din], BF16, kind="ExternalInput").ap()
    w1 = nc.dram_tensor("w1", [c.din, c.dh], BF16, kind="ExternalInput").ap()
    w2 = nc.dram_tensor("w2", [c.dh, c.dh2], F32, kind="ExternalInput").ap()
    b1r = nc.dram_tensor("b1r", [128, c.dh], F32, kind="ExternalInput").ap()
    b2r = nc.dram_tensor("b2r", [128, c.dh2], F32, kind="ExternalInput").ap()
    degp = nc.dram_tensor("degp", [128, c.nt], F32, kind="ExternalInput").ap()
    idx16 = nc.dram_tensor("idx16", [128, tot_slots // 16], I16,
                           kind="ExternalInput").ap()
    idsf = nc.dram_tensor("idsf", [128, TC], BF16, kind="ExternalInput").ap()
    out_h = nc.dram_tensor("out_h", [c.nloc, c.dh2], F32, kind="ExternalOutput").ap()
    out_ls = nc.dram_tensor("out_ls", [c.nloc, c.dh2], F32, kind="ExternalOutput").ap()

    t1_loc = nc.dram_tensor("t1_loc", [c.nloc, c.dh], BF16, kind="Internal").ap()
    t1_full = nc.dram_tensor(
        "t1_full", [c.cores * c.nloc, c.dh], BF16, kind="Internal",
        addr_space="Shared"
    ).ap()
    t2_loc = nc.dram_tensor("t2_loc", [c.nloc, c.dt2], BF16, kind="Internal").ap()
    t2_full = nc.dram_tensor(
        "t2_full", [c.cores * c.nloc, c.dt2], BF16, kind="Internal",
        addr_space="Shared"
    ).ap()

    with ExitStack() as st:
        cpool = st.enter_context(tc.tile_pool(name="consts", bufs=1))
        accp = st.enter_context(tc.tile_pool(name="acc", bufs=1))
        gp = st.enter_context(tc.tile_pool(name="gp", bufs=2))
        sp = st.enter_context(tc.tile_pool(name="sp", bufs=2))
        pp = st.enter_context(tc.tile_pool(name="pp", bufs=2))
        ppsum = st.enter_context(tc.tile_pool(name="ppsum", bufs=4, space="PSUM"))
        p0 = st.enter_context(tc.tile_pool(name="p0", bufs=3))
        p0ps = st.enter_context(tc.tile_pool(name="p0ps", bufs=2, space="PSUM"))
        p0psT = st.enter_context(tc.tile_pool(name="p0psT", bufs=2, space="PSUM"))

        # ---- constants ----
        ident = cpool.tile([128, 128], F32)
        make_identity(nc, ident)
        identb = cpool.tile([128, 128], BF16)
        make_identity(nc, identb)
        w1sb = cpool.tile([128, c.kt, c.dh], BF16)
        nc.sync.dma_start(w1sb, w1.rearrange("(o p) f -> p o f", p=128))
        w2sb = cpool.tile([128, c.dh2], F32)
        nc.sync.dma_start(w2sb, w2)
        b1sb = cpool.tile([128, c.dh], F32)
        nc.sync.dma_start(b1sb, b1r)
        b2sb = cpool.tile([128, c.dh2], F32)
        nc.sync.dma_start(b2sb, b2r)
        dinv = cpool.tile([128, c.nt], F32)
        nc.sync.dma_start(dinv, degp)
        nc.scalar.activation(dinv, dinv, AF.Sqrt)
        nc.vector.reciprocal(dinv, dinv)
        iota = cpool.tile([128, MAXP, 256], BF16)
        nc.gpsimd.iota(iota, pattern=[[0, MAXP], [1, 256]], base=0,
                       channel_multiplier=0,
                       allow_small_or_imprecise_dtypes=True)

        # ---- phase 0: T1 = dinv * (x @ W1) ----
        for t in range(c.nt):
            xt = p0.tile([128, c.din], BF16, tag="xt")
            nc.sync.dma_start(xt, x_sh[ts(t, 128), :])
            hps = p0ps.tile([128, c.dh], F32, tag="hps")
            for j in range(c.kt):
                tps = p0psT.tile([128, 128], BF16, tag="tps")
                nc.tensor.transpose(tps, xt[:, ts(j, 128)], identb)
                xT = p0.tile([128, 128], BF16, tag="xT")
                nc.vector.tensor_copy(xT, tps)
                nc.tensor.matmul(
                    hps, lhsT=xT, rhs=w1sb[:, j, :],
                    start=(j == 0), stop=(j == c.kt - 1),
                )
            hsb = p0.tile([128, c.dh], BF16, tag="hsb")
            nc.vector.tensor_scalar_mul(hsb, hps, dinv[:, t : t + 1])
            nc.sync.dma_start(t1_loc[ts(t, 128), :], hsb)
        nc.gpsimd.collective_compute(
            "AllGather", ALU.bypass, replica_groups=RG,
            ins=[t1_loc.opt()], outs=[t1_full.opt()],
        )

        # ---- edge aggregation ----
        def edge_phase(tfull, tloc, acc, d, dt):
            # self-loop seed: acc[:, b, :] = T[b-th block rows]
            tv = tloc.rearrange("(b p) f -> p b f", p=128)
            for t in range(c.nt):
                sd = p0.tile([128, dt], BF16, tag="sd")
                nc.sync.dma_start(sd[:, :d], tv[:, t, :d])
                nc.vector.tensor_copy(acc[:, t, :], sd[:, :d])
            chunk0 = 0
            psum_open = {}
            for w in range(W):
                nch_w = plan.nchunks[w]
                sit = sp.tile([128, maxch * 8], I16, tag="sit")
                nc.sync.dma_start(
                    sit[:, : nch_w * 8],
                    idx16[:, chunk0 * 8 : (chunk0 + nch_w) * 8],
                )
                sid = sp.tile([128, maxch], BF16, tag="sid")
                nc.sync.dma_start(sid[:, :nch_w], idsf[:, chunk0 : chunk0 + nch_w])
                loc = 0
                while loc < nch_w:
                    nch = min(MAXP, nch_w - loc)
                    g = gp.tile([128, MAXP, dt], BF16, tag="gt")
                    nc.gpsimd.dma_gather(
                        g[:, :nch, :], tfull[ds(w * 2 * c.nloc, 2 * c.nloc), :],
                        sit[:, loc * 8 : (loc + nch) * 8],
                        num_idxs=nch * 128, num_idxs_reg=nch * 128, elem_size=dt,
                        single_packet=False, queue_num=3,
                    )
                    stt = pp.tile([128, MAXP, 256], BF16, tag="stt")
                    nc.vector.tensor_tensor(
                        stt[:, :nch, :], iota[:, :nch, :],
                        sid[:, loc : loc + nch, None].to_broadcast((128, nch, 256)),
                        ALU.is_equal,
                    )
                    for j in range(nch):
                        for (b, par, k0, k1, start, stop) in plan.chunk_ops[w][loc + j]:
                            if start:
                                pbt = ppsum.tile([128, d], F32, tag="ps")
                                psum_open[b] = pbt
                            nc.tensor.matmul(
                                psum_open[b],
                                lhsT=stt[k0:k1, j, ds(par * 128, 128)],
                                rhs=g[k0:k1, j, :d],
                                start=start, stop=stop,
                            )
                            if stop:
                                nc.vector.tensor_tensor(
                                    acc[:, b, :], acc[:, b, :], psum_open[b],
                                    ALU.add,
                                )
                                del psum_open[b]
                    loc += nch
                chunk0 += nch_w
            assert not psum_open

        acc1 = accp.tile([128, c.nt, c.dh], F32)
        edge_phase(t1_full, t1_loc, acc1, c.dh, c.dh)

        # ---- g1 = dinv * relu(dinv * agg + b1), per quarter so phase 2
        # can start before the layer-1 tail finishes ----
        for qi in range(4):
            q0, qn = c.qstart[qi], c.qtiles[qi]
            a1 = acc1[:, q0 : q0 + qn, :]
            dbc = dinv[:, q0 : q0 + qn, None].to_broadcast((128, qn, c.dh))
            nc.vector.tensor_tensor(a1, a1, dbc, ALU.mult)
            nc.vector.tensor_tensor(
                a1, a1, b1sb[:, None, :].to_broadcast((128, qn, c.dh)), ALU.add
            )
            nc.scalar.activation(a1, a1, AF.Relu)
            nc.vector.tensor_tensor(a1, a1, dbc, ALU.mult)

        # ---- phase 2: T2 = g1 @ W2 ----
        for t in range(c.nt):
            tps = p0psT.tile([128, 128], F32, tag="tps")
            nc.tensor.transpose(tps, acc1[:, t, :], ident)
            gT = p0.tile([128, 128], F32, tag="gT")
            nc.vector.tensor_copy(gT, tps)
            h2ps = p0ps.tile([128, c.dh2], F32, tag="hps")
            nc.tensor.matmul(h2ps, lhsT=gT, rhs=w2sb, start=True, stop=True)
            h2sb = p0.tile([128, c.dh2], BF16, tag="h2sb")
            nc.vector.tensor_copy(h2sb, h2ps)
            nc.sync.dma_start(t2_loc[ts(t, 128), : c.dh2], h2sb)
            nc.sync.dma_start(t2_loc[ts(t, 128), c.dh2 :], h2sb)
        nc.gpsimd.collective_compute(
            "AllGather", ALU.bypass, replica_groups=RG,
            ins=[t2_loc.opt()], outs=[t2_full.opt()],
        )

        # ---- layer-2 edge aggregation ----
        acc2 = accp.tile([128, c.nt, c.dh2], F32)
        edge_phase(t2_full, t2_loc, acc2, c.dh2, c.dt2)

        # ---- h = dinv * agg2 + b2 ; log_softmax (batched) ----
        ohv = out_h.rearrange("(t p) f -> p t f", p=128)
        olv = out_ls.rearrange("(t p) f -> p t f", p=128)
        nc.vector.tensor_tensor(
            acc2, acc2, dinv[:, :, None].to_broadcast((128, c.nt, c.dh2)), ALU.mult
        )
        nc.vector.tensor_tensor(
            acc2, acc2, b2sb[:, None, :].to_broadcast((128, c.nt, c.dh2)), ALU.add
        )
        nc.sync.dma_start(ohv, acc2)
        accN = acc2[:, :, : c.dout]
        mx = accp.tile([128, c.nt], F32, tag="mx")
        nc.vector.tensor_reduce(mx, accN, mybir.AxisListType.X, ALU.max)
        nc.vector.tensor_tensor(
            accN, accN, mx[:, :, None].to_broadcast((128, c.nt, c.dout)),
            ALU.subtract,
        )
        e1 = accp.tile([128, c.nt, c.dout], F32, tag="e1")
        nc.scalar.activation(e1, accN, AF.Exp)
        se = accp.tile([128, c.nt], F32, tag="se")
        nc.vector.tensor_reduce(se, e1, mybir.AxisListType.X, ALU.add)
        ln = accp.tile([128, c.nt], F32, tag="ln")
        nc.scalar.activation(ln, se, AF.Ln)
        nc.vector.tensor_tensor(
            accN, accN, ln[:, :, None].to_broadcast((128, c.nt, c.dout)),
            ALU.subtract,
        )
        nc.sync.dma_start(olv[:, :, : c.dout], accN)


# ----------------------------------------------------------------------------
# Host entry point
# ----------------------------------------------------------------------------

_CACHE = {}


def _get_compiled(cfg: Cfg, plan: Plan):
    key = (cfg, plan)
    if key not in _CACHE:
        nc = bacc.Bacc(
            "TRN2", target_bir_lowering=False, debug=False,
            num_devices=cfg.cores, num_swdge_queues=4,
        )
        with tile.TileContext(nc) as tc:
            build(nc, tc, cfg, plan)
        nc.compile()
        _CACHE[key] = nc
    return _CACHE[key]


def make_in_maps(cfg: Cfg, x, W1, b1, W2, b2, deg_pt, idx16, sidf):
    import ml_dtypes

    c = cfg
    x = np.asarray(x, np.float32)
    w2p = np.zeros((c.dh, c.dh2), np.float32)
    w2p[:, : c.dout] = np.asarray(W2, np.float32)
    b1rep = np.tile(np.asarray(b1, np.float32)[None, :], (128, 1))
    b2p = np.zeros(c.dh2, np.float32)
    b2p[: c.dout] = np.asarray(b2, np.float32)
    b2rep = np.tile(b2p[None, :], (128, 1))
    w1c = np.ascontiguousarray(
        np.asarray(W1, np.float32).astype(ml_dtypes.bfloat16)
    )

    in_maps = []
    for ci in range(c.cores):
        xs = np.zeros((c.nloc, c.din), ml_dtypes.bfloat16)
        xs[: c.nsh] = x[ci * c.nsh : (ci + 1) * c.nsh].astype(ml_dtypes.bfloat16)
        in_maps.append({
            "x_sh": xs,
            "w1": w1c,
            "w2": w2p,
            "b1r": b1rep,
            "b2r": b2rep,
            "degp": np.ascontiguousarray(deg_pt[ci]),
            "idx16": np.ascontiguousarray(idx16[ci]),
            "idsf": np.ascontiguousarray(sidf[ci].astype(ml_dtypes.bfloat16)),
        })
    return in_maps


def _ensure_ntff_hook():
    """Install the axon NTFF profile hook if the image's antenv lacks it."""
    import types

    try:
        from antenv.axon_hooks import get_axon_ntff_profile_hook  # noqa: F401
        return
    except ImportError:
        pass
    import antenv

    m = types.ModuleType("antenv.axon_hooks")
    m._hook = None
    m.set_axon_ntff_profile_hook = lambda h: setattr(m, "_hook", h)
    m.get_axon_ntff_profile_hook = lambda: m._hook
    sys.modules["antenv.axon_hooks"] = m
    antenv.axon_hooks = m
    try:
        from trn_agent_boot.trn_boot import _ntff_profile_via_ctypes

        h = _ntff_profile_via_ctypes("/opt/axon/libaxon_pjrt.so")
        if h is not None:
            m._hook = h
    except Exception as e:
        print(f"ntff hook install failed: {e}")

    from concourse import bass_utils as bu

    bu.upload_artifacts = lambda tmpdir: tmpdir


def run(cfg: Cfg, inputs: dict, trace: bool = False):
    if trace:
        _ensure_ntff_hook()
    deg_pt, idx16, sidf, plan = preprocess(cfg, inputs["edge_index"])
    nc = _get_compiled(cfg, plan)
    in_maps = make_in_maps(
        cfg, inputs["x"], inputs["W1"], inputs["b1"], inputs["W2"], inputs["b2"],
        deg_pt, idx16, sidf,
    )
    res = run_bass_kernel_spmd(
        nc, in_maps, core_ids=list(range(cfg.cores)), trace=trace
    )
    c = cfg
    h = np.concatenate(
        [res.results[ci]["out_h"][: c.nsh, : c.dout] for ci in range(c.cores)],
        axis=0,
    )
    ls = np.concatenate(
        [res.results[ci]["out_ls"][: c.nsh, : c.dout] for ci in range(c.cores)],
        axis=0,
    )
    return (h, ls), res


def kernel(**inputs):
    (h, ls), _ = run(Cfg(), inputs)
    return h, ls


# revision 19
# speedup vs baseline: 1.1870x; 1.0041x over previous
"""Trainium2 Bass kernel for a 2-layer GCN (nn_EvenLamerGCN) — v3.

reference semantics (PyG GCNConv x2, eval mode):
    deg[i]  = 1 + indeg(i)                (self-loops added)
    dinv    = deg ** -0.5
    h  = relu(A_hat @ (x @ W1) + b1),  A_hat = D^-1/2 (A + I) D^-1/2
    o  = A_hat @ (h @ W2) + b2
    return o, log_softmax(o, axis=1)

v3 = v1 (SWDGE dma_gather rows + one-hot scatter matmuls) with:
  * 64-granular cell quotas (was 128-chunk granularity): per-(window,
    block) slot quotas are mult-of-64 maxima over cores; chunks of 128
    slots may straddle two cells, handled by partial-row matmuls
    (base partition 0/64) and parity-offset slot ids (S is built
    256 wide; even blocks match columns 0-127, odd blocks 128-255).
  * 4 equal-ish windows defined by LOCAL row quarters, each with its
    own t_loc/t_full tensors and its own AllGather, so collectives
    pipeline against phase-0 tails and gather processing.
  * self-loops seeded into acc from the local table shard (no
    descriptors).
"""

import sys

for _p in ("/opt/trn_rl_repo", "/root/.axon_site/_ro/trn_rl_repo"):
    if _p not in sys.path:
        sys.path.insert(0, _p)

from contextlib import ExitStack
from dataclasses import dataclass

import numpy as np

import concourse.bass as bass
import concourse.mybir as mybir
import concourse.tile as tile
from concourse import bacc
from concourse.bass import ds, ts
from concourse.bass_utils import run_bass_kernel_spmd
from concourse.masks import make_identity

F32 = mybir.dt.float32
BF16 = mybir.dt.bfloat16
I16 = mybir.dt.int16
AF = mybir.ActivationFunctionType
ALU = mybir.AluOpType

MAXP = 32            # chunks per gather instruction


@dataclass(frozen=True)
class Cfg:
    n: int = 100000
    din: int = 512
    dh: int = 128
    dout: int = 40
    cores: int = 8

    @property
    def nsh(self):
        return self.n // self.cores

    @property
    def nloc(self):
        return ((self.nsh + 127) // 128) * 128

    @property
    def nt(self):
        return self.nloc // 128

    @property
    def dh2(self):
        return 64

    @property
    def dt2(self):
        return 128

    @property
    def kt(self):
        return self.din // 128

    @property
    def qtiles(self):
        # phase-0 tile counts per window-quarter (sums to nt)
        q, r = divmod(self.nt, 4)
        return tuple(q + (1 if i < r else 0) for i in range(4))

    @property
    def qstart(self):
        s, out = 0, []
        for q in self.qtiles:
            out.append(s)
            s += q
        return tuple(out)

    @property
    def qrows(self):
        return tuple(q * 128 for q in self.qtiles)


@dataclass(frozen=True)
class Plan:
    q: tuple          # [w][b] slot quota, mult of 64
    nchunks: tuple    # chunks per window
    chunk_ops: tuple  # [w][chunk] -> ((b, k0, k1, start, stop), ...)

    @property
    def total_chunks(self):
        return sum(self.nchunks)


def make_plan(cfg: Cfg, counts: np.ndarray) -> Plan:
    c = cfg
    W = len(c.qtiles)
    qmax = counts.max(axis=0)                # [W, nt]
    q = ((qmax + 63) // 64) * 64
    nchunks = []
    chunk_ops = []
    for w in range(W):
        sw = int(q[w].sum())
        nch = (sw + 127) // 128
        nchunks.append(nch)
        ops = [[] for _ in range(nch)]
        cur = 0
        ordinal = 0
        for b in range(c.nt):
            qb = int(q[w][b])
            if qb == 0:
                continue
            par = ordinal & 1
            ordinal += 1
            lo, hi = cur, cur + qb
            cur = hi
            c0, c1 = lo // 128, (hi - 1) // 128
            for ci in range(c0, c1 + 1):
                k0 = max(lo, ci * 128) - ci * 128
                k1 = min(hi, (ci + 1) * 128) - ci * 128
                assert k0 in (0, 64), (w, b, lo, hi, ci)
                ops[ci].append((b, par, k0, k1, ci == c0, ci == c1))
        chunk_ops.append(tuple(tuple(x) for x in ops))
    return Plan(
        q=tuple(tuple(int(v) for v in row) for row in q),
        nchunks=tuple(nchunks),
        chunk_ops=tuple(chunk_ops),
    )


def preprocess(cfg: Cfg, edge_index: np.ndarray):
    c = cfg
    W = len(c.qtiles)
    src = np.asarray(edge_index[0], dtype=np.int64)
    dst = np.asarray(edge_index[1], dtype=np.int64)

    deg = np.bincount(dst, minlength=c.n).astype(np.float32) + 1.0
    deg_pt = np.ones((c.cores, 128, c.nt), np.float32)
    for ci in range(c.cores):
        dl = np.ones(c.nloc, np.float32)
        dl[: c.nsh] = deg[ci * c.nsh : (ci + 1) * c.nsh]
        deg_pt[ci] = dl.reshape(c.nt, 128).T

    core = dst // c.nsh
    dloc = dst - core * c.nsh
    b_all = dloc // 128
    id_all = dloc % 128
    csrc = src // c.nsh
    sloc = src - csrc * c.nsh
    w_all = csrc // 2
    r_all = (csrc % 2) * c.nloc + sloc

    cell_key = (core * W + w_all) * c.nt + b_all
    counts = np.bincount(cell_key, minlength=c.cores * W * c.nt)
    counts = counts.reshape(c.cores, W, c.nt)
    plan = make_plan(c, counts)

    tot_slots = plan.total_chunks * 128
    TC = plan.total_chunks

    idx16 = np.zeros((c.cores, 128, tot_slots // 16), np.int16)
    sidf = np.empty((c.cores, 128, TC), np.float32)

    order = np.lexsort((r_all, b_all, w_all, core))
    so_r, so_b, so_id = r_all[order], b_all[order], id_all[order]
    so_core = core[order]
    core_starts = np.searchsorted(so_core, np.arange(c.cores + 1))

    wbase = np.concatenate([[0], np.cumsum([n * 128 for n in plan.nchunks])])
    cell_off = np.zeros((W, c.nt), np.int64)
    cell_par = np.zeros((W, c.nt), np.int64)
    for w in range(W):
        cur = 0
        ordinal = 0
        for b in range(c.nt):
            cell_off[w][b] = cur
            if plan.q[w][b]:
                cell_par[w][b] = ordinal & 1
                ordinal += 1
            cur += plan.q[w][b]

    for ci in range(c.cores):
        lo, hi = core_starts[ci], core_starts[ci + 1]
        rr, ii = so_r[lo:hi], so_id[lo:hi]
        vals = np.zeros(tot_slots, np.int64)
        sids = np.full(tot_slots, -1.0, np.float32)
        cnt = counts[ci]
        pos = 0
        for w in range(W):
            for b in range(c.nt):
                n = cnt[w][b]
                if n:
                    off = wbase[w] + cell_off[w][b]
                    vals[off : off + n] = rr[pos : pos + n]
                    sids[off : off + n] = ii[pos : pos + n] + 128 * cell_par[w][b]
                    pos += n
        assert pos == hi - lo
        v = vals.reshape(-1, 16)
        idx16[ci] = np.tile(np.ascontiguousarray(v.T), (8, 1)).astype(np.int16)
        sidf[ci] = sids.reshape(TC, 128).T

    return deg_pt, idx16, sidf, plan


# ----------------------------------------------------------------------------
# Device kernel
# ----------------------------------------------------------------------------

def build(nc, tc, cfg: Cfg, plan: Plan):
    c = cfg
    RG = [list(range(c.cores))]
    W = len(c.qtiles)
    TC = plan.total_chunks
    tot_slots = TC * 128
    maxch = max(plan.nchunks)

    x_sh = nc.dram_tensor("x_sh", [c.nloc, c.din], BF16, kind="ExternalInput").ap()
    w1 = nc.dram_tensor("w1", [c.din, c.dh], BF16, kind="ExternalInput").ap()
    w2 = nc.dram_tensor("w2", [c.dh, c.dh2], F32, kind="ExternalInput").ap()
    b1r = nc.dram_tensor("b1r", [128, c.dh], F32, kind="ExternalInput").ap()
    b2r = nc.dram_tensor("b2r", [128, c.dh2], F32, kind="ExternalInput").ap()
    degp = nc.dram_tensor("degp", [128, c.nt], F32, kind="ExternalInput").ap()
    idx16 = nc.dram_tensor("idx16", [128, tot_slots // 16], I16,
                           kind="ExternalInput").ap()
    idsf = nc.dram_tensor("idsf", [128, TC], BF16, kind="ExternalInput").ap()
    out_h = nc.dram_tensor("out_h", [c.nloc, c.dh2], F32, kind="ExternalOutput").ap()
    out_ls = nc.dram_tensor("out_ls", [c.nloc, c.dh2], F32, kind="ExternalOutput").ap()

    t1_loc = nc.dram_tensor("t1_loc", [c.nloc, c.dh], BF16, kind="Internal").ap()
    t1_full = nc.dram_tensor(
        "t1_full", [c.cores * c.nloc, c.dh], BF16, kind="Internal",
        addr_space="Shared"
    ).ap()
    t2_loc = nc.dram_tensor("t2_loc", [c.nloc, c.dt2], BF16, kind="Internal").ap()
    t2_full = nc.dram_tensor(
        "t2_full", [c.cores * c.nloc, c.dt2], BF16, kind="Internal",
        addr_space="Shared"
    ).ap()

    with ExitStack() as st:
        cpool = st.enter_context(tc.tile_pool(name="consts", bufs=1))
        accp = st.enter_context(tc.tile_pool(name="acc", bufs=1))
        gp = st.enter_context(tc.tile_pool(name="gp", bufs=2))
        sp = st.enter_context(tc.tile_pool(name="sp", bufs=2))
        pp = st.enter_context(tc.tile_pool(name="pp", bufs=2))
        ppsum = st.enter_context(tc.tile_pool(name="ppsum", bufs=4, space="PSUM"))
        p0 = st.enter_context(tc.tile_pool(name="p0", bufs=3))
        p0ps = st.enter_context(tc.tile_pool(name="p0ps", bufs=2, space="PSUM"))
        p0psT = st.enter_context(tc.tile_pool(name="p0psT", bufs=2, space="PSUM"))

        # ---- constants ----
        ident = cpool.tile([128, 128], F32)
        make_identity(nc, ident)
        identb = cpool.tile([128, 128], BF16)
        make_identity(nc, identb)
        w1sb = cpool.tile([128, c.kt, c.dh], BF16)
        nc.sync.dma_start(w1sb, w1.rearrange("(o p) f -> p o f", p=128))
        w2sb = cpool.tile([128, c.dh2], F32)
        nc.sync.dma_start(w2sb, w2)
        b1sb = cpool.tile([128, c.dh], F32)
        nc.sync.dma_start(b1sb, b1r)
        b2sb = cpool.tile([128, c.dh2], F32)
        nc.sync.dma_start(b2sb, b2r)
        dinv = cpool.tile([128, c.nt], F32)
        nc.sync.dma_start(dinv, degp)
        nc.scalar.activation(dinv, dinv, AF.Sqrt)
        nc.vector.reciprocal(dinv, dinv)
        iota = cpool.tile([128, MAXP, 256], BF16)
        nc.gpsimd.iota(iota, pattern=[[0, MAXP], [1, 256]], base=0,
                       channel_multiplier=0,
                       allow_small_or_imprecise_dtypes=True)

        # ---- phase 0: T1 = dinv * (x @ W1) ----
        for t in range(c.nt):
            xt = p0.tile([128, c.din], BF16, tag="xt")
            nc.sync.dma_start(xt, x_sh[ts(t, 128), :])
            hps = p0ps.tile([128, c.dh], F32, tag="hps")
            for j in range(c.kt):
                tps = p0psT.tile([128, 128], BF16, tag="tps")
                nc.tensor.transpose(tps, xt[:, ts(j, 128)], identb)
                xT = p0.tile([128, 128], BF16, tag="xT")
                nc.vector.tensor_copy(xT, tps)
                nc.tensor.matmul(
                    hps, lhsT=xT, rhs=w1sb[:, j, :],
                    start=(j == 0), stop=(j == c.kt - 1),
                )
            hsb = p0.tile([128, c.dh], BF16, tag="hsb")
            nc.vector.tensor_scalar_mul(hsb, hps, dinv[:, t : t + 1])
            nc.sync.dma_start(t1_loc[ts(t, 128), :], hsb)
        nc.gpsimd.collective_compute(
            "AllGather", ALU.bypass, replica_groups=RG,
            ins=[t1_loc.opt()], outs=[t1_full.opt()],
        )

        # ---- edge aggregation ----
        def edge_phase(tfull, tloc, acc, d, dt):
            # self-loop seed: acc[:, b, :] = T[b-th block rows]
            tv = tloc.rearrange("(b p) f -> p b f", p=128)
            for t in range(c.nt):
                sd = p0.tile([128, dt], BF16, tag="sd")
                nc.sync.dma_start(sd[:, :d], tv[:, t, :d])
                nc.vector.tensor_copy(acc[:, t, :], sd[:, :d])
            chunk0 = 0
            psum_open = {}
            for w in range(W):
                nch_w = plan.nchunks[w]
                sit = sp.tile([128, maxch * 8], I16, tag="sit")
                nc.sync.dma_start(
                    sit[:, : nch_w * 8],
                    idx16[:, chunk0 * 8 : (chunk0 + nch_w) * 8],
                )
                sid = sp.tile([128, maxch], BF16, tag="sid")
                nc.sync.dma_start(sid[:, :nch_w], idsf[:, chunk0 : chunk0 + nch_w])
                loc = 0
                while loc < nch_w:
                    nch = min(MAXP, nch_w - loc)
                    g = gp.tile([128, MAXP, dt], BF16, tag="gt")
                    nc.gpsimd.dma_gather(
                        g[:, :nch, :], tfull[ds(w * 2 * c.nloc, 2 * c.nloc), :],
                        sit[:, loc * 8 : (loc + nch) * 8],
                        num_idxs=nch * 128, num_idxs_reg=nch * 128, elem_size=dt,
                        single_packet=False, queue_num=3,
                    )
                    stt = pp.tile([128, MAXP, 256], BF16, tag="stt")
                    nc.vector.tensor_tensor(
                        stt[:, :nch, :], iota[:, :nch, :],
                        sid[:, loc : loc + nch, None].to_broadcast((128, nch, 256)),
                        ALU.is_equal,
                    )
                    for j in range(nch):
                        for (b, par, k0, k1, start, stop) in plan.chunk_ops[w][loc + j]:
                            if start:
                                pbt = ppsum.tile([128, d], F32, tag="ps")
                                psum_open[b] = pbt
                            nc.tensor.matmul(
                                psum_open[b],
                                lhsT=stt[k0:k1, j, ds(par * 128, 128)],
                                rhs=g[k0:k1, j, :d],
                                start=start, stop=stop,
                            )
                            if stop:
                                nc.vector.tensor_tensor(
                                    acc[:, b, :], acc[:, b, :], psum_open[b],
                                    ALU.add,
                                )
                                del psum_open[b]
                    loc += nch
                chunk0 += nch_w
            assert not psum_open

        acc1 = accp.tile([128, c.nt, c.dh], F32)
        edge_phase(t1_full, t1_loc, acc1, c.dh, c.dh)

        # ---- g1 = dinv * relu(dinv * agg + b1), per quarter so phase 2
        # can start before the layer-1 tail finishes ----
        for qi in range(4):
            q0, qn = c.qstart[qi], c.qtiles[qi]
            a1 = acc1[:, q0 : q0 + qn, :]
            dbc = dinv[:, q0 : q0 + qn, None].to_broadcast((128, qn, c.dh))
            nc.vector.tensor_tensor(a1, a1, dbc, ALU.mult)
            nc.vector.tensor_tensor(
                a1, a1, b1sb[:, None, :].to_broadcast((128, qn, c.dh)), ALU.add
            )
            nc.scalar.activation(a1, a1, AF.Relu)
            nc.vector.tensor_tensor(a1, a1, dbc, ALU.mult)

        # ---- phase 2: T2 = g1 @ W2 ----
        for t in range(c.nt):
            tps = p0psT.tile([128, 128], F32, tag="tps")
            nc.tensor.transpose(tps, acc1[:, t, :], ident)
            gT = p0.tile([128, 128], F32, tag="gT")
            nc.vector.tensor_copy(gT, tps)
            h2ps = p0ps.tile([128, c.dh2], F32, tag="hps")
            nc.tensor.matmul(h2ps, lhsT=gT, rhs=w2sb, start=True, stop=True)
            h2sb = p0.tile([128, c.dh2], BF16, tag="h2sb")
            nc.vector.tensor_copy(h2sb, h2ps)
            nc.sync.dma_start(t2_loc[ts(t, 128), : c.dh2], h2sb)
            nc.sync.dma_start(t2_loc[ts(t, 128), c.dh2 :], h2sb)
        nc.gpsimd.collective_compute(
            "AllGather", ALU.bypass, replica_groups=RG,
            ins=[t2_loc.opt()], outs=[t2_full.opt()],
        )

        # ---- layer-2 edge aggregation ----
        acc2 = accp.tile([128, c.nt, c.dh2], F32)
        edge_phase(t2_full, t2_loc, acc2, c.dh2, c.dt2)

        # ---- h = dinv * agg2 + b2 ; log_softmax, per quarter so the
        # output tail overlaps the layer-2 gather stream ----
        ohv = out_h.rearrange("(t p) f -> p t f", p=128)
        olv = out_ls.rearrange("(t p) f -> p t f", p=128)
        for qi in range(4):
            q0, qn = c.qstart[qi], c.qtiles[qi]
            a2 = acc2[:, q0 : q0 + qn, :]
            nc.vector.tensor_tensor(
                a2, a2,
                dinv[:, q0 : q0 + qn, None].to_broadcast((128, qn, c.dh2)),
                ALU.mult,
            )
            nc.vector.tensor_tensor(
                a2, a2, b2sb[:, None, :].to_broadcast((128, qn, c.dh2)), ALU.add
            )
            nc.sync.dma_start(ohv[:, q0 : q0 + qn, :], a2)
            accN = a2[:, :, : c.dout]
            mx = accp.tile([128, qn], F32, tag=f"mx{qi}")
            nc.vector.tensor_reduce(mx, accN, mybir.AxisListType.X, ALU.max)
            nc.vector.tensor_tensor(
                accN, accN, mx[:, :, None].to_broadcast((128, qn, c.dout)),
                ALU.subtract,
            )
            e1 = accp.tile([128, qn, c.dout], F32, tag=f"e1{qi}")
            nc.scalar.activation(e1, accN, AF.Exp)
            se = accp.tile([128, qn], F32, tag=f"se{qi}")
            nc.vector.tensor_reduce(se, e1, mybir.AxisListType.X, ALU.add)
            ln = accp.tile([128, qn], F32, tag=f"ln{qi}")
            nc.scalar.activation(ln, se, AF.Ln)
            nc.vector.tensor_tensor(
                accN, accN, ln[:, :, None].to_broadcast((128, qn, c.dout)),
                ALU.subtract,
            )
            nc.sync.dma_start(olv[:, q0 : q0 + qn, : c.dout], accN)


# ----------------------------------------------------------------------------
# Host entry point
# ----------------------------------------------------------------------------

_CACHE = {}


def _get_compiled(cfg: Cfg, plan: Plan):
    key = (cfg, plan)
    if key not in _CACHE:
        nc = bacc.Bacc(
            "TRN2", target_bir_lowering=False, debug=False,
            num_devices=cfg.cores, num_swdge_queues=4,
        )
        with tile.TileContext(nc) as tc:
            build(nc, tc, cfg, plan)
        nc.compile()
        _CACHE[key] = nc
    return _CACHE[key]


def make_in_maps(cfg: Cfg, x, W1, b1, W2, b2, deg_pt, idx16, sidf):
    import ml_dtypes

    c = cfg
    x = np.asarray(x, np.float32)
    w2p = np.zeros((c.dh, c.dh2), np.float32)
    w2p[:, : c.dout] = np.asarray(W2, np.float32)
    b1rep = np.tile(np.asarray(b1, np.float32)[None, :], (128, 1))
    b2p = np.zeros(c.dh2, np.float32)
    b2p[: c.dout] = np.asarray(b2, np.float32)
    b2rep = np.tile(b2p[None, :], (128, 1))
    w1c = np.ascontiguousarray(
        np.asarray(W1, np.float32).astype(ml_dtypes.bfloat16)
    )

    in_maps = []
    for ci in range(c.cores):
        xs = np.zeros((c.nloc, c.din), ml_dtypes.bfloat16)
        xs[: c.nsh] = x[ci * c.nsh : (ci + 1) * c.nsh].astype(ml_dtypes.bfloat16)
        in_maps.append({
            "x_sh": xs,
            "w1": w1c,
            "w2": w2p,
            "b1r": b1rep,
            "b2r": b2rep,
            "degp": np.ascontiguousarray(deg_pt[ci]),
            "idx16": np.ascontiguousarray(idx16[ci]),
            "idsf": np.ascontiguousarray(sidf[ci].astype(ml_dtypes.bfloat16)),
        })
    return in_maps


def _ensure_ntff_hook():
    """Install the axon NTFF profile hook if the image's antenv lacks it."""
    import types

    try:
        from antenv.axon_hooks import get_axon_ntff_profile_hook  # noqa: F401
        return
    except ImportError:
        pass
    import antenv

    m = types.ModuleType("antenv.axon_hooks")
    m._hook = None
    m.set_axon_ntff_profile_hook = lambda h: setattr(m, "_hook", h)
    m.get_axon_ntff_profile_hook = lambda: m._hook
    sys.modules["antenv.axon_hooks"] = m
    antenv.axon_hooks = m
    try:
        from trn_agent_boot.trn_boot import _ntff_profile_via_ctypes

        h = _ntff_profile_via_ctypes("/opt/axon/libaxon_pjrt.so")
        if h is not None:
            m._hook = h
    except Exception as e:
        print(f"ntff hook install failed: {e}")

    from concourse import bass_utils as bu

    bu.upload_artifacts = lambda tmpdir: tmpdir


def run(cfg: Cfg, inputs: dict, trace: bool = False):
    if trace:
        _ensure_ntff_hook()
    deg_pt, idx16, sidf, plan = preprocess(cfg, inputs["edge_index"])
    nc = _get_compiled(cfg, plan)
    in_maps = make_in_maps(
        cfg, inputs["x"], inputs["W1"], inputs["b1"], inputs["W2"], inputs["b2"],
        deg_pt, idx16, sidf,
    )
    res = run_bass_kernel_spmd(
        nc, in_maps, core_ids=list(range(cfg.cores)), trace=trace
    )
    c = cfg
    h = np.concatenate(
        [res.results[ci]["out_h"][: c.nsh, : c.dout] for ci in range(c.cores)],
        axis=0,
    )
    ls = np.concatenate(
        [res.results[ci]["out_ls"][: c.nsh, : c.dout] for ci in range(c.cores)],
        axis=0,
    )
    return (h, ls), res


def kernel(**inputs):
    (h, ls), _ = run(Cfg(), inputs)
    return h, ls
